# revision 66
# baseline (speedup 1.0000x reference)
"""Causal self-attention (B=8, T=1024, C=768, H=8 heads) for 8 TRN2 NeuronCores.

Strategy: pure data parallelism - one batch element per core, no collectives.

Numerics plan (rel-err budget 2e-2; emulated end-to-end error ~= 0.0105):
  - q/k projection runs in fp8 e4m3 with DoubleRow perf mode (K=256 per
    instruction, 2x PE throughput). x and W_qk are quantized host-side with a
    32x scale on W (and 32x on the q/k copyback), folded out via the exp scale.
  - S = K^T Q also runs in fp8 DoubleRow by splitting the head dim 96 into two
    48-row k-tiles living side by side in the free axis of qT8/kT8.
  - V path and P@V run in bf16; out-projection in fp32r (full-rate).

Layouts (host-prepped, all uploads already transposed/packed/quantized):
  xt8  [128, 6144] e4m3 : xt8[p, cb*1024+t] = x[t, cb*128+p]          (qk proj)
  xtbf [128, 6144] bf16 : xtbf[p, t*6+cb]   = x[t, cb*128+p]          (v proj,
         token-major so the DMA can be chunked by token block)
  wqk8 [128, 10752] e4m3: DoubleRow-padded: per cb-pair pi, k-tile i, group
         g=(qk*8+h): cols [g*112+0:48]=W[:,qk*768+h*96+0:48]*32,
         [g*112+64:112]=W[...48:96]*32, 16 zero cols between, so the matmul
         output lands on psum partitions 0:48 / 64:112 (legal engine starts).
  wv   [128, 4608] bf16 ; wp [128, 4608] f32 : cb-major blocks of 768 cols.
  bqk  [112, 16] f32    : 32*b_attn packed to match the padded psum rows.

Per-head pipeline (emission order S(h) -> qkproj(h+1) -> PV(h) -> tail(h)):
  S^T[tk,q] blocks via fp8 DR; exp on ACT -> p_t bf16; diagonal-block
  triangle mask via DVE/GpSimd (alternating); P@V with bf16 v_aug (extra ones
  column produces the softmax denominator in row 96); denominator reciprocal
  on DVE (reciprocal_approx_fast); K=1 PE outer-product broadcasts the recip
  row to [96,512]; y_n = y * bc scattered into feature-major yTp via DMA;
  final projection yTp @ wp in fp32r.
"""
import sys

sys.path.insert(0, "/opt/trn_rl_repo")

import numpy as np
import ml_dtypes

T, C, H, D = 1024, 768, 8, 96
C3 = 3 * C
P = 128
NT = T // P   # 8 token blocks
NCB = C // P  # 6 feature blocks
NPR = NCB // 2  # 3 cb pairs for DoubleRow
DA = D + 1    # 97: head dim + denominator column
DH = D // 2   # 48: half head dim (S DoubleRow k-tile)
WQ = 32.0     # fp8 scale on W_qk (and q/k copyback bias)

E4NP = ml_dtypes.float8_e4m3
BFNP = ml_dtypes.bfloat16

DEBUG_TAPS = False

_CACHE = {}


def _build(with_bias=False):
    import concourse.bacc as bacc
    import concourse.mybir as mybir
    import concourse.tile as tile

    F32 = mybir.dt.float32
    F32R = mybir.dt.float32r
    BF16 = mybir.dt.bfloat16
    F8 = mybir.dt.float8e4
    DR = mybir.MatmulPerfMode.DoubleRow
    Exp = mybir.ActivationFunctionType.Exp
    Copy = mybir.ActivationFunctionType.Copy
    is_ge = mybir.AluOpType.is_ge
    SCALE = 1.0 / (float(np.sqrt(D)) * WQ * WQ)

    nc = bacc.Bacc("TRN2", target_bir_lowering=False, debug=False, num_devices=8)
    xt8_d = nc.dram_tensor("xt8", [P, NCB * T], F8, kind="ExternalInput").ap()
    xtbf_d = nc.dram_tensor("xtbf", [P, NCB * T], BF16, kind="ExternalInput").ap()
    wqk8_d = nc.dram_tensor("wqk8", [P, NPR * 2 * 1536], F8,
                            kind="ExternalInput").ap()
    wv_d = nc.dram_tensor("wv", [P, NCB * C], BF16, kind="ExternalInput").ap()
    wp_d = nc.dram_tensor("wp", [P, NCB * C], BF16, kind="ExternalInput").ap()
    bqk_d = nc.dram_tensor("bqk", [D, 16], F32, kind="ExternalInput").ap()
    bv_d = nc.dram_tensor("bv", [C], BF16, kind="ExternalInput").ap()
    bp_d = nc.dram_tensor("bp", [C], F32, kind="ExternalInput").ap()
    out_d = nc.dram_tensor("out", [T, C], F32, kind="ExternalOutput").ap()
    dbg = {}
    if DEBUG_TAPS:
        dbg["q"] = nc.dram_tensor("dbg_q", [D, T], F8, kind="ExternalOutput").ap()
        dbg["k"] = nc.dram_tensor("dbg_k", [D, T], F8, kind="ExternalOutput").ap()
        dbg["p"] = nc.dram_tensor("dbg_p", [P, T], BF16, kind="ExternalOutput").ap()
        dbg["p2"] = nc.dram_tensor("dbg_p2", [P, T], BF16, kind="ExternalOutput").ap()
        dbg["rc"] = nc.dram_tensor("dbg_rc", [1, 512], F32, kind="ExternalOutput").ap()
        dbg["bc"] = nc.dram_tensor("dbg_bc", [D, 512], F32, kind="ExternalOutput").ap()
        dbg["yn"] = nc.dram_tensor("dbg_yn", [D, 512], BF16, kind="ExternalOutput").ap()
        dbg["va"] = nc.dram_tensor("dbg_va", [P, DA * H], BF16, kind="ExternalOutput").ap()

    with tile.TileContext(nc) as tc:
        with tc.tile_pool(name="const", bufs=1) as const_p, \
             tc.tile_pool(name="xw", bufs=1) as xw_p, \
             tc.tile_pool(name="vp", bufs=1) as v_p, \
             tc.tile_pool(name="qkt", bufs=4) as qk_p, \
             tc.tile_pool(name="yt", bufs=1) as yT_p, \
             tc.tile_pool(name="pt", bufs=10) as p_p, \
             tc.tile_pool(name="sm", bufs=4) as sm_p, \
             tc.tile_pool(name="ob", bufs=2) as o_p, \
             tc.tile_pool(name="ps", bufs=1, space="PSUM") as ps:
            # ---- constants ----
            tri_f = const_p.tile([P, P], F32, name="tri_f")
            tri = const_p.tile([P, P], BF16, name="tri")
            ones8 = const_p.tile([P, H], BF16, name="ones8")
            ones96f = const_p.tile([1, D], F32, name="ones96f")
            ones96 = const_p.tile([1, D], F32R, name="ones96")
            nc.vector.memset(ones8, 1.0)
            nc.vector.memset(ones96f, 1.0)
            nc.vector.tensor_copy(ones96[:], ones96f[:])
            bqk = const_p.tile([D, 16], F32, name="bqk")
            bv_bc = const_p.tile([P, C], BF16, name="bv_bc")
            bp_bc = const_p.tile([P, C], F32, name="bp_bc")
            # lower-left triangle mask: tri[tk, u] = 1.0 iff u >= tk
            nc.gpsimd.memset(tri_f, 1.0)
            nc.gpsimd.affine_select(
                out=tri_f, in_=tri_f, compare_op=is_ge, fill=0.0,
                base=0, pattern=[[1, P]], channel_multiplier=-1)
            nc.vector.tensor_copy(tri, tri_f)
            # DRAM staging for the reciprocal rows (SBUF->SBUF DMA cannot do
            # 0-stride broadcast, DRAM->SBUF can)
            rc_dram = nc.dram_tensor("rc_stage", [2 * H, 512], F32,
                                     kind="Internal").ap()

            # ---- input/weight DMAs, spread across engine queues so the
            # transfers start in parallel at t=0 ----
            wv = xw_p.tile([P, NCB * C], BF16, name="wv")
            xtbf = xw_p.tile([P, NCB * T], BF16, name="xtbf")
            xt8 = xw_p.tile([P, NCB * T], F8, name="xt8")
            wqk8 = xw_p.tile([P, NPR * 2 * 1536], F8, name="wqk8")
            wp = xw_p.tile([P, NCB * C], BF16, name="wp")
            # wv + the first xtbf chunk gate the first matmul: spread them
            # over the two fast (HW-DGE) rings; bulk fp8 goes via gpsimd
            nc.sync.dma_start(wv[:, 0:2304], wv_d[:, 0:2304])
            nc.scalar.dma_start(wv[:, 2304:4608], wv_d[:, 2304:4608])
            nc.sync.dma_start(xtbf[:, 0:1536], xtbf_d[:, 0:1536])
            nc.scalar.dma_start(xt8[:], xt8_d)
            nc.gpsimd.dma_start(wqk8[:], wqk8_d)
            nc.gpsimd.dma_start(bqk[:], bqk_d)
            nc.gpsimd.dma_start(
                bv_bc[:], bv_d.unsqueeze(0).partition_broadcast(P).squeeze(1))
            nc.sync.dma_start(xtbf[:, 1536:3072], xtbf_d[:, 1536:3072])
            nc.scalar.dma_start(xtbf[:, 3072:4608], xtbf_d[:, 3072:4608])
            nc.sync.dma_start(xtbf[:, 4608:6144], xtbf_d[:, 4608:6144])
            nc.gpsimd.dma_start(
                bp_bc[:], bp_d.unsqueeze(0).partition_broadcast(P).squeeze(1))
            nc.sync.dma_start(wp[:], wp_d)

            # rearranged views
            xtbf_r = xtbf[:].rearrange("p (t c) -> p t c", c=NCB)
            xt8_r = xt8[:].rearrange("p (pi i t) -> p pi i t", pi=NPR, i=2)
            wqk8_r = wqk8[:].rearrange("p (pi i g) -> p pi i g", pi=NPR, i=2)
            # g axis = original W_attn column (q cols 0:768 | k cols 768:1536)

            # ---- v projection: vA[tb] = [v | ones] per head, bf16 ----
            vA = [v_p.tile([P, DA * H], BF16, name=f"vA{t}") for t in range(NT)]
            for tb in range(NT):
                # ones columns at local col 96 of each head's group
                nc.vector.tensor_copy(vA[tb][:, D::DA], ones8)
            yTp = [yT_p.tile([P, T], BF16, name=f"yTp{cb}") for cb in range(NCB)]
            for tb in range(NT):
                v_ps = ps.tile([P, C], F32, name="v_ps", tag="s", bufs=2)
                for cb in range(NCB):
                    lhsT = xtbf_r[:, tb * P:(tb + 1) * P, cb]
                    nc.tensor.matmul(v_ps[:, 0:512], lhsT,
                                     wv[:, cb * C:cb * C + 512],
                                     start=(cb == 0), stop=(cb == NCB - 1))
                    nc.tensor.matmul(v_ps[:, 512:C], lhsT,
                                     wv[:, cb * C + 512:(cb + 1) * C],
                                     start=(cb == 0), stop=(cb == NCB - 1))
                # single strided add: [p, h, d] += bias, into the 97-strided vA
                nc.vector.tensor_add(
                    vA[tb][:].rearrange("p (h da) -> p h da", da=DA)[:, :, 0:D],
                    v_ps[:].rearrange("p (h d) -> p h d", d=D),
                    bv_bc[:].rearrange("p (h d) -> p h d", d=D))
                if DEBUG_TAPS and tb == 0:
                    nc.sync.dma_start(dbg["va"], vA[0][:])

            # ---- q/k projection for one head: fp8 DoubleRow (K=256) ----
            def emit_qkproj(h):
                qT = qk_p.tile([D, T], F8, name="qT", tag="qkt")
                kT = qk_p.tile([D, T], F8, name="kT", tag="qkt")
                for qk, dst in ((0, qT), (1, kT)):
                    g = qk * 8 + h
                    base = qk * C + h * D
                    bcol = bqk[:, g:g + 1]
                    for jt in range(2):
                        qk_ps = ps.tile([D, 512], F32, name="qk_ps",
                                        tag="qky", bufs=4)
                        for pi in range(NPR):
                            nc.tensor.matmul(
                                qk_ps[:],
                                wqk8_r[:, pi, :, base:base + D],
                                xt8_r[:, pi, :, jt * 512:(jt + 1) * 512],
                                start=(pi == 0), stop=(pi == NPR - 1),
                                perf_mode=DR)
                        nc.vector.tensor_scalar_add(
                            dst[:, jt * 512:(jt + 1) * 512], qk_ps[:], bcol)
                if DEBUG_TAPS and h == 0:
                    nc.sync.dma_start(dbg["q"], qT[:])
                    nc.sync.dma_start(dbg["k"], kT[:])
                return qT, kT

            qkT = emit_qkproj(0)

            def emit_tail(h, y_l, y_r, bcs):
                # normalize y by the broadcast reciprocal rows and scatter
                # into the feature-packed yT tiles (partition shift -> DMA).
                # Emitted one head late so the broadcast round-trip latency
                # never blocks the DVE queue.
                for half, y_ps in ((0, y_l), (1, y_r)):
                    y_n = sm_p.tile([D, 512], BF16, name="y_n", tag="yn",
                                    bufs=4)
                    nc.vector.tensor_mul(y_n[:], y_ps[0:D, :], bcs[half][:])
                    if DEBUG_TAPS and h == 0 and half == 0:
                        nc.sync.dma_start(dbg["yn"], y_n[:])
                    q_sl = slice(half * 512, (half + 1) * 512)
                    f0 = D * h
                    while f0 < D * (h + 1):
                        cb2, b0 = f0 // P, f0 % P
                        seg = min(P - b0, D * (h + 1) - f0)
                        nc.sync.dma_start(
                            yTp[cb2][b0:b0 + seg, q_sl],
                            y_n[f0 - D * h:f0 - D * h + seg, :])
                        f0 += seg

            pending = None

            # ---- per-head attention loop ----
            for h in range(H):
                qT, kT = qkT
                ptiles = []
                for ib in range(NT):
                    q0 = P * ib
                    s_ps = ps.tile([P, T], F32, name="s_ps", tag="s", bufs=2)
                    kblk = kT[:, ib * P:(ib + 1) * P]
                    if q0 < 512:
                        nc.tensor.matmul(s_ps[:, q0:512], kblk,
                                         qT[:, q0:512], start=True, stop=True)
                    r0 = max(q0, 512)
                    nc.tensor.matmul(s_ps[:, r0:T], kblk, qT[:, r0:T],
                                     start=True, stop=True)
                    p_t = p_p.tile([P, T], BF16, name="p_t")
                    nc.scalar.activation(p_t[:, q0:T], s_ps[:, q0:T],
                                         Exp, scale=SCALE)
                    # zero the upper triangle of the diagonal 128-col block
                    # all-SBUF bf16 -> legal on gpsimd, keeps DVE free
                    nc.gpsimd.tensor_mul(p_t[:, q0:q0 + P], p_t[:, q0:q0 + P],
                                         tri)
                    if DEBUG_TAPS and h == 0 and ib <= 1:
                        nc.sync.dma_start(dbg["p"] if ib == 0 else dbg["p2"],
                                          p_t[:])
                    ptiles.append(p_t)

                if h + 1 < H:
                    qkT = emit_qkproj(h + 1)

                # P@V: two bank-halves of q, each its own accumulation group
                y_l = ps.tile([DA, 512], F32, name="y_l", tag="qky", bufs=4)
                y_r = ps.tile([DA, 512], F32, name="y_r", tag="qky", bufs=4)
                rc_l = sm_p.tile([1, 512], F32, name="rc_l", tag="rc", bufs=4)
                rc_r = sm_p.tile([1, 512], F32, name="rc_r", tag="rc", bufs=4)
                dn_l = sm_p.tile([1, 512], F32, name="dn_l", tag="dn", bufs=4)
                dn_r = sm_p.tile([1, 512], F32, name="dn_r", tag="dn", bufs=4)
                for ib in range(NT):
                    q0 = P * ib
                    va = vA[ib][:, DA * h:DA * h + DA]
                    if q0 < 512:
                        nc.tensor.matmul(y_l[:, q0:512], va,
                                         ptiles[ib][:, q0:512],
                                         start=(ib == 0), stop=(ib == 3))
                        nc.tensor.matmul(y_r[:], va, ptiles[ib][:, 512:T],
                                         start=(ib == 0), stop=False)
                    else:
                        nc.tensor.matmul(y_r[:, q0 - 512:512], va,
                                         ptiles[ib][:, q0:T],
                                         start=False, stop=(ib == NT - 1))
                    if ib == 3:
                        # y_l complete: reciprocal of its denominator row now,
                        # so the broadcast below doesn't stall.
                        # (reciprocal_approx_fast misbehaves on PSUM inputs ->
                        # bounce the row through SBUF first)
                        nc.vector.tensor_copy(dn_l[:], y_l[D:DA, :])
                        nc.vector.reciprocal_approx_fast(
                            out=rc_l[:], in_=dn_l[:])
                nc.vector.tensor_copy(dn_r[:], y_r[D:DA, :])
                nc.vector.reciprocal_approx_fast(out=rc_r[:], in_=dn_r[:])

                # broadcast the recip rows to [96, 512] via a DRAM round-trip
                bcs = []
                for half, rc in ((0, rc_l), (1, rc_r)):
                    bc_sb = sm_p.tile([D, 512], F32, name="bc_sb", tag="bcs",
                                      bufs=4)
                    if h < H - 1:
                        # broadcast the recip row via a DRAM round-trip; the
                        # latency hides behind the one-head-late tail
                        row = rc_dram[2 * h + half:2 * h + half + 1, :]
                        nc.sync.dma_start(row, rc[:])
                        nc.sync.dma_start(
                            bc_sb[:], row.partition_broadcast(D).squeeze(1))
                    else:
                        # last head: nothing hides the round-trip, so build
                        # the broadcast on-chip with a K=1 outer product on
                        # the now-idle PE (psum "s" slots are free too), then
                        # bounce to SBUF (tensor_tensor can't read two PSUMs)
                        rcr = sm_p.tile([1, 512], F32R, name="rcr", tag="rcr",
                                        bufs=2)
                        nc.vector.tensor_copy(rcr[:], rc[:])
                        bc_ps = ps.tile([D, 512], F32, name="bc_ps", tag="s",
                                        bufs=2)
                        nc.tensor.matmul(bc_ps[:], ones96[:], rcr[:],
                                         start=True, stop=True)
                        nc.vector.tensor_copy(bc_sb[:], bc_ps[:])
                    if DEBUG_TAPS and h == 0 and half == 0:
                        nc.sync.dma_start(dbg["rc"], rc[:])
                        nc.sync.dma_start(dbg["bc"], bc_sb[:])
                    bcs.append(bc_sb)

                if pending is not None:
                    emit_tail(*pending)
                pending = (h, y_l, y_r, bcs)
            emit_tail(*pending)

            # ---------------- projection ----------------
            for tb in range(NT):
                o_ps = ps.tile([P, C], F32, name="o_ps", tag="s", bufs=2)
                for cb in range(NCB):
                    nc.tensor.matmul(o_ps[:, 0:512],
                                     yTp[cb][:, tb * P:(tb + 1) * P],
                                     wp[:, cb * C:cb * C + 512],
                                     start=(cb == 0), stop=(cb == NCB - 1))
                o_sb = o_p.tile([P, C], F32, name="o_sb")
                for cb in range(NCB):
                    nc.tensor.matmul(o_ps[:, 512:C],
                                     yTp[cb][:, tb * P:(tb + 1) * P],
                                     wp[:, cb * C + 512:(cb + 1) * C],
                                     start=(cb == 0), stop=(cb == NCB - 1))
                # zero-bias fast path: copyback on ACT (idle here), out DMAs
                # on the scalar queue so the stores use an empty DMA ring
                for half, sl in enumerate((slice(0, 512), slice(512, C))):
                    if with_bias:
                        nc.vector.tensor_add(o_sb[:, sl], o_ps[:, sl],
                                             bp_bc[:, sl])
                    else:
                        nc.scalar.activation(o_sb[:, sl], o_ps[:, sl], Copy)
                    # alternate store rings so the 3.1MB of output drains at
                    # 2x single-ring bandwidth
                    eng = nc.sync if half == 0 else nc.scalar
                    eng.dma_start(out_d[tb * P:(tb + 1) * P, sl], o_sb[:, sl])

    nc.compile()
    return nc


def _prep_weights(wa, ba, wpj, bp):
    """Host-side packing/quantization of the shared weights."""
    # q/k weights: fp8 e4m3, 32x scale; c = pi*256 + i*128 + p
    wqk8 = (wa[:, :2 * C] * WQ).astype(E4NP).reshape(NPR, 2, P, 2 * C)
    wqk8 = np.ascontiguousarray(wqk8.transpose(2, 0, 1, 3)).reshape(
        P, NPR * 2 * 1536)
    wv = np.ascontiguousarray(
        wa[:, 2 * C:].reshape(NCB, P, C).transpose(1, 0, 2)
    ).reshape(P, NCB * C).astype(BFNP)
    wpk = np.ascontiguousarray(
        wpj.reshape(NCB, P, C).transpose(1, 0, 2)
    ).reshape(P, NCB * C).astype(BFNP)
    bq = (ba[:2 * C] * WQ).astype(np.float32).reshape(2, H, D)
    bqk = np.ascontiguousarray(bq.transpose(2, 0, 1)).reshape(D, 16)
    bv = ba[2 * C:].astype(BFNP)
    return {"wqk8": wqk8, "wv": wv, "wp": wpk, "bqk": bqk, "bv": bv,
            "bp": bp.astype(np.float32)}


def run(inputs, trace=False):
    import concourse.bass_utils as bass_utils

    x = np.ascontiguousarray(inputs["x"], dtype=np.float32)
    wa = np.ascontiguousarray(inputs["W_attn"], dtype=np.float32)
    ba = np.ascontiguousarray(inputs["b_attn"], dtype=np.float32)
    wpj = np.ascontiguousarray(inputs["W_proj"], dtype=np.float32)
    bp = np.ascontiguousarray(inputs["b_proj"], dtype=np.float32)
    with_bias = bool(ba.any() or bp.any())
    nc = _CACHE.get(("nc", with_bias))
    if nc is None:
        nc = _CACHE[("nc", with_bias)] = _build(with_bias)
    B = x.shape[0]
    shared = _prep_weights(wa, ba, wpj, bp)
    in_maps = []
    for b in range(B):
        xt = np.ascontiguousarray(x[b].T)                     # [768, 1024]
        blk = xt.reshape(NCB, P, T)
        xt8 = np.ascontiguousarray(blk.transpose(1, 0, 2)).reshape(
            P, NCB * T).astype(E4NP)
        xtbf = np.ascontiguousarray(blk.transpose(1, 2, 0)).reshape(
            P, NCB * T).astype(BFNP)
        in_maps.append({"xt8": xt8, "xtbf": xtbf, **shared})
    res = bass_utils.run_bass_kernel_spmd(
        nc, in_maps, core_ids=list(range(B)), trace=trace)
    out = np.stack([r["out"] for r in res.results], axis=0)
    return out, res


def kernel(**inputs):
    out, _ = run(inputs, trace=False)
    return out
